# revision 6
# baseline (speedup 1.0000x reference)
"""GRU + MLP head kernel for Trainium2 — v5: re-associated recurrence.

Single chain (batch 32). The recurrence handoff is the delta
Delta(t) = zb*(ht - h): since W@h(t+1) = W@h(t) + W@Delta(t), the 8 rz
matmuls with rhs=h(t) run OFF the critical path (during step t), and only
the 8 rz matmuls with rhs=Delta(t) plus h(t+1)=h(t)+Delta(t) remain, with
the h-add itself off-path. Per-step on-path:
  Delta -> 8 rz-delta-matmuls -> sigmoid[r|zb] -> rh -> 4 n-matmuls -> tanh
        -> u=ht-h -> Delta'=zb*u

Projections matmul directly into PSUM per 8-step window (6 regions x 256
cols = 3 banks, double buffered). z-gate weights/bias negated on host so
sigmoid gives zb = 1-z directly.
Regions: 0,1 = r (m0,m1); 2,3 = -z; 4,5 = n.
"""

import numpy as np
import ml_dtypes

BF = ml_dtypes.bfloat16
P = 128
B, T, I, H = 256, 512, 128, 256
NCORES = 8
BL = B // NCORES
WIN = 8
NW = T // WIN
CW = WIN * BL  # 256

_prog_cache = {}


def _build_program(reps=1, with_bias=False, wh_early=True, wbufs=3, split_sig=False):
    import concourse.bacc as bacc
    import concourse.tile as tile
    from concourse.tile_rust import add_dep_helper
    from concourse import mybir

    f32 = mybir.dt.float32
    bf16 = mybir.dt.bfloat16
    AF = mybir.ActivationFunctionType

    nc = bacc.Bacc("TRN2", target_bir_lowering=False, debug=False)

    xT_d = nc.dram_tensor("xT", [P, T * BL], bf16, kind="ExternalInput")
    whrz_d = nc.dram_tensor("whrz", [P, 8 * P], bf16, kind="ExternalInput")
    whrz8_d = nc.dram_tensor("whrz8", [P, 8 * P], mybir.dt.float8e4,
                             kind="ExternalInput")
    whn_d = nc.dram_tensor("whn", [P, 4 * P], bf16, kind="ExternalInput")
    wx_d = nc.dram_tensor("wx", [P, 6 * P], bf16, kind="ExternalInput")
    wb_d = nc.dram_tensor("wb", [1, 6 * P], bf16, kind="ExternalInput")
    w1T_d = nc.dram_tensor("w1T", [P, 2 * 64], bf16, kind="ExternalInput")
    w2T_d = nc.dram_tensor("w2T", [64, 10], bf16, kind="ExternalInput")
    b1_d = nc.dram_tensor("b1", [64, 1], f32, kind="ExternalInput")
    b2_d = nc.dram_tensor("b2", [1, 10], bf16, kind="ExternalInput")
    out_d = nc.dram_tensor("out", [BL, 10], f32, kind="ExternalOutput")

    with tile.TileContext(nc) as tc:
        with (
            tc.tile_pool(name="consts", bufs=1) as consts,
            tc.tile_pool(name="xp", bufs=1) as xp,
            tc.tile_pool(name="state", bufs=1) as state,
            tc.tile_pool(name="work", bufs=wbufs) as work,
            tc.tile_pool(name="wps", bufs=2, space="PSUM") as wps,
            tc.tile_pool(name="hps", bufs=1, space="PSUM") as hps,
        ):
            whrz_sb = consts.tile([P, 8 * P], bf16)
            nc.sync.dma_start(out=whrz_sb, in_=whrz_d.ap())
            whrz8_sb = consts.tile([P, 8 * P], mybir.dt.float8e4)
            nc.sync.dma_start(out=whrz8_sb, in_=whrz8_d.ap())
            whn_sb = consts.tile([P, 4 * P], bf16)
            nc.sync.dma_start(out=whn_sb, in_=whn_d.ap())
            wx_sb = consts.tile([P, 6 * P], bf16)
            nc.sync.dma_start(out=wx_sb, in_=wx_d.ap())
            wb_sb = consts.tile([1, 6 * P], bf16)
            nc.sync.dma_start(out=wb_sb, in_=wb_d.ap())
            w1T_sb = consts.tile([P, 2 * 64], bf16)
            nc.sync.dma_start(out=w1T_sb, in_=w1T_d.ap())
            w2T_sb = consts.tile([64, 10], bf16)
            nc.sync.dma_start(out=w2T_sb, in_=w2T_d.ap())
            b1_sb = consts.tile([64, 1], f32)
            nc.sync.dma_start(out=b1_sb, in_=b1_d.ap())
            b2_sb = consts.tile([1, 10], bf16)
            nc.sync.dma_start(out=b2_sb, in_=b2_d.ap())
            ones_sb = consts.tile([1, CW], bf16)
            nc.vector.memset(ones_sb, 1.0)

            def emit_body(_iv=None):
                xT_sb = xp.tile([P, T * BL], bf16, tag="xT", name="xT")
                # chunked load: the first projection window only waits for
                # the first 512KB chunk; the rest streams under the
                # recurrence.
                NCHUNK = 8
                CCOLS = T * BL // NCHUNK
                for ci in range(NCHUNK):
                    nc.sync.dma_start(
                        out=xT_sb[:, ci * CCOLS : (ci + 1) * CCOLS],
                        in_=xT_d.ap()[:, ci * CCOLS : (ci + 1) * CCOLS])

                h_tiles = [state.tile([P, 2 * BL], bf16, tag=f"h{i}", name=f"h{i}")
                           for i in range(2)]
                nc.vector.memset(h_tiles[0], 0.0)

                def emit_proj(w):
                    ps = wps.tile([P, 6 * CW], f32, tag="wps", name=f"wps{w}")
                    rhs = xT_sb[:, w * CW : (w + 1) * CW]
                    prev = None
                    for reg in range(6):
                        mm = nc.tensor.matmul(
                            ps[:, reg * CW : (reg + 1) * CW],
                            wx_sb[:, reg * P : (reg + 1) * P],
                            rhs,
                            start=(reg % 2 == 0), stop=False,
                            skip_group_check=True,
                        )
                        if reg % 2 == 1:
                            add_dep_helper(mm.ins, prev.ins, sync=False,
                                           reason="bank clear order")
                        prev = mm
                        if with_bias:
                            nc.tensor.matmul(
                                ps[:, reg * CW : (reg + 1) * CW],
                                wb_sb[:, reg * P : (reg + 1) * P],
                                ones_sb,
                                start=False, stop=False,
                                skip_group_check=True,
                            )
                    return ps, ps.rearrange("p (r s b) -> p r s b", r=6, s=WIN)

                def mm_grp(ps, s, regs, rhs, wsb, woff):
                    for i, reg in enumerate(regs):
                        for k in range(2):
                            nc.tensor.matmul(
                                ps[:, reg * CW + s * BL : reg * CW + (s + 1) * BL],
                                wsb[:, (woff + i * 2 + k) * P : (woff + i * 2 + k + 1) * P],
                                rhs[:, k * BL : (k + 1) * BL],
                                start=False, stop=(k == 1),
                                skip_group_check=True,
                            )

                d_prev = [None]

                def emit_step(pp, pp_next, s, t):
                    ps, psv = pp
                    h_in = h_tiles[t % 2]      # h(t)
                    h_out = h_tiles[(t + 1) % 2]
                    wt = lambda nm, fd=2 * BL: work.tile([P, fd], bf16, tag=nm,
                                                         name=f"{nm}_{t}")
                    # --- on-path: rz delta-matmuls (skipped at t=0: psum is
                    # proj + Wh@h(0) with h(0)=0) ---
                    if split_sig:
                        if d_prev[0] is not None:
                            mm_grp(ps, s, (0, 1), d_prev[0], whrz_sb, 0)
                        rzb = wt("rzb", 4 * BL)
                        nc.scalar.activation(rzb[:, 0 : 2 * BL],
                                             psv[:, 0:2, s, :], AF.Sigmoid)
                        rh = wt("rh")
                        nc.vector.tensor_mul(rh, rzb[:, 0 : 2 * BL], h_in)
                        # z-half after rh so rh's ACT wait threshold is sig_r
                        if d_prev[0] is not None:
                            mm_grp(ps, s, (2, 3), d_prev[0], whrz_sb, 4)
                        nc.scalar.activation(rzb[:, 2 * BL : 4 * BL],
                                             psv[:, 2:4, s, :], AF.Sigmoid)
                    else:
                        if d_prev[0] is not None:
                            mm_grp(ps, s, (0, 1, 2, 3), d_prev[0], whrz8_sb, 0)
                        rzb = wt("rzb", 4 * BL)
                        nc.scalar.activation(rzb, psv[:, 0:4, s, :], AF.Sigmoid)
                        rh = wt("rh")
                        nc.vector.tensor_mul(rh, rzb[:, 0 : 2 * BL], h_in)

                    def emit_wh():
                        # off-path: accumulate Wh@h(t) into step t+1's psum.
                        # Must come after the sigmoid (bank-hazard ordering).
                        if t + 1 < T:
                            s2 = (t + 1) % WIN
                            ps2 = ps if s2 != 0 else pp_next[0]
                            mm_grp(ps2, s2, (0, 1, 2, 3), h_in, whrz_sb, 0)

                    if wh_early:
                        emit_wh()
                    # --- on-path: n matmuls -> tanh -> u -> Delta ---
                    mm_grp(ps, s, (4, 5), rh, whn_sb, 0)
                    if not wh_early:
                        emit_wh()
                    ht = wt("ht")
                    nc.scalar.activation(ht, psv[:, 4:6, s, :], AF.Tanh)
                    u = wt("u")
                    nc.vector.tensor_sub(u, ht, h_in)
                    d = wt("d")
                    nc.vector.tensor_mul(d, rzb[:, 2 * BL : 4 * BL], u)
                    # --- off-path: h(t+1) = h(t) + Delta(t) ---
                    nc.vector.tensor_add(h_out, h_in, d)
                    d_prev[0] = d

                pp = emit_proj(0)
                pp_next = None
                for t in range(T):
                    if t % WIN == 0 and t + WIN < T:
                        pp_next = emit_proj(t // WIN + 1)
                    emit_step(pp, pp_next, t % WIN, t)
                    if (t + 1) % WIN == 0 and t + 1 < T:
                        pp = pp_next

                t = T
                # ---- head ----
                h_fin = h_tiles[t % 2]
                sh = work.tile([P, 2 * BL], bf16, tag="sh", name="sh")
                nc.scalar.activation(sh, h_fin, AF.Sigmoid)
                p1 = hps.tile([64, BL], f32, tag="p1", name="p1")
                for k in range(2):
                    nc.tensor.matmul(
                        p1, w1T_sb[:, k * 64 : (k + 1) * 64],
                        sh[:, k * BL : (k + 1) * BL],
                        start=(k == 0), stop=(k == 1), skip_group_check=True,
                    )
                s1 = work.tile([64, BL], bf16, tag="s1", name="s1")
                nc.scalar.activation(s1, p1, AF.Sigmoid, bias=b1_sb)
                p2 = hps.tile([BL, 10], f32, tag="p2", name="p2")
                nc.tensor.matmul(p2, ones_sb[:, 0:BL], b2_sb, start=True,
                                 stop=False, skip_group_check=True)
                nc.tensor.matmul(p2, s1, w2T_sb, start=False, stop=True,
                                 skip_group_check=True)
                lg = work.tile([BL, 10], f32, tag="lg", name="lg")
                nc.vector.tensor_copy(lg, p2)
                mx = work.tile([BL, 1], f32, tag="mx", name="mx")
                nc.vector.reduce_max(mx, lg, axis=mybir.AxisListType.X)
                nmx = work.tile([BL, 1], f32, tag="nmx", name="nmx")
                nc.vector.tensor_scalar_mul(nmx, mx, -1.0)
                ex = work.tile([BL, 10], f32, tag="ex", name="ex")
                nc.scalar.activation(ex, lg, AF.Exp, bias=nmx)
                sm = work.tile([BL, 1], f32, tag="sm", name="sm")
                nc.vector.reduce_sum(sm, ex, axis=mybir.AxisListType.X)
                ri = work.tile([BL, 1], f32, tag="ri", name="ri")
                nc.vector.reciprocal(ri, sm)
                oo = work.tile([BL, 10], f32, tag="oo", name="oo")
                nc.vector.tensor_scalar_mul(oo, ex, ri)
                nc.sync.dma_start(out=out_d.ap(), in_=oo)

            if reps == 1:
                emit_body()
            else:
                with tc.For_i(0, reps, 1) as _iv:
                    emit_body(_iv)

    nc.compile()
    return nc


def _get_program(reps=1, with_bias=False, wh_early=True, wbufs=3, split_sig=False):
    key = (reps, with_bias, wh_early, wbufs, split_sig)
    if key not in _prog_cache:
        _prog_cache[key] = _build_program(reps, with_bias, wh_early, wbufs,
                                          split_sig)
    return _prog_cache[key]


def _pack_inputs(x, weight_xr, weight_hr, bias_r, weight_xz, weight_hz, bias_z,
             weight_x, weight_h, bias, w1, b1, w2, b2):
    def blkT(W, m, k, neg=False):
        blk = W[m * P : (m + 1) * P, k * P : (k + 1) * P].T
        if neg:
            blk = -blk
        return np.ascontiguousarray(blk.astype(BF))

    whrz = np.concatenate(
        [blkT(weight_hr, m, k) for m in range(2) for k in range(2)]
        + [blkT(weight_hz, m, k, neg=True) for m in range(2) for k in range(2)],
        axis=1)
    whn = np.concatenate(
        [blkT(weight_h, m, k) for m in range(2) for k in range(2)], axis=1)

    def xblkT(W, m, neg=False):
        blk = W[m * P : (m + 1) * P, :].T
        if neg:
            blk = -blk
        return np.ascontiguousarray(blk.astype(BF))

    wx = np.concatenate(
        [xblkT(weight_xr, 0), xblkT(weight_xr, 1),
         xblkT(weight_xz, 0, True), xblkT(weight_xz, 1, True),
         xblkT(weight_x, 0), xblkT(weight_x, 1)], axis=1)
    wb = np.concatenate(
        [bias_r[:P], bias_r[P:], -bias_z[:P], -bias_z[P:],
         bias[:P], bias[P:]]).reshape(1, 6 * P).astype(BF)

    w1T = np.concatenate(
        [np.ascontiguousarray(w1[:, k * P : (k + 1) * P].T.astype(BF))
         for k in range(2)], axis=1)
    w2T = np.ascontiguousarray(w2.T.astype(BF))
    b1c = b1.reshape(64, 1).astype(np.float32)
    b2c = b2.reshape(1, 10).astype(BF)

    whrz8 = whrz.astype(ml_dtypes.float8_e4m3)
    common = dict(whrz=whrz, whrz8=whrz8, whn=whn, wx=wx, wb=wb,
                  w1T=w1T, w2T=w2T, b1=b1c, b2=b2c)
    in_maps = []
    for c in range(NCORES):
        xs = x[c * BL : (c + 1) * BL].astype(BF)          # [BL, T, I]
        xTc = np.ascontiguousarray(xs.transpose(2, 1, 0)  # [I, T, BL]
                                   .reshape(P, T * BL))
        in_maps.append(dict(common, xT=xTc))
    return in_maps


def kernel(x, weight_xr, weight_hr, bias_r, weight_xz, weight_hz, bias_z,
           weight_x, weight_h, bias, w1, b1, w2, b2):
    from concourse.bass_utils import run_bass_kernel_spmd

    x = np.asarray(x, np.float32)
    args = [np.asarray(a, np.float32) for a in
            (weight_xr, weight_hr, bias_r, weight_xz, weight_hz, bias_z,
             weight_x, weight_h, bias, w1, b1, w2, b2)]
    zero_bias = all(np.all(a == 0) for a in (args[2], args[5], args[8]))
    nc = _get_program(reps=1, with_bias=not zero_bias)
    in_maps = _pack_inputs(x, *args)
    res = run_bass_kernel_spmd(nc, in_maps, core_ids=list(range(NCORES)))
    return np.concatenate([res.results[c]["out"] for c in range(NCORES)], axis=0)
